# revision 23
# baseline (speedup 1.0000x reference)
"""Trainium2 Bass kernel for nn_BDFM_Multi (B=8,C=256,H=W=128,N=4).

Data-parallel over batch: one batch element per NeuronCore (8 cores).

Per-core computation (feature f [C,HW], m [N,H,W], HW=16384):
  z    = (m > 0.3)                                  binary
  er   = 13-tap separable min-filter(z), dl = 13-tap separable max-filter(z)
         -> banded 0/1 matmuls + thresholds (exact on binary data)
  fbu  = per-class channels (er, 1-dl, dl-er)       [12, HW]
  mid  = fbu @ f^T                                  [12, C]
  A'^T = mid @ Wo2'^T, Y = mid @ Wf', v = mid @ beta_f
  X    = Y^T @ A'^T  (= (G Wf')^T = Wf'^T G^T)
  Wc^T = Wo1'^T + X;  u = A'^T^T v + beta_o
  out  = Wc @ f + u                                 [C, HW]
  (exact algebraic refactor of out = BN(Wo @ [f; mid^T @ (mid @ BN(Wf@f))]))

v3: bf16 feature/out streams; transposes staggered around morphology with a
fully persistent f^T buffer; k-major fbu^T built directly from PSUM; the
small [C,C] stage runs in 7 engine hops (float32r).
"""
import numpy as np
import ml_dtypes
from contextlib import ExitStack

import concourse.bass as bass
import concourse.mybir as mybir
import concourse.tile as tile
from concourse import bacc
from concourse import bass_utils

F32 = mybir.dt.float32
F32R = mybir.dt.float32r
BF16 = mybir.dt.bfloat16
ALU = mybir.AluOpType
ACTF = mybir.ActivationFunctionType

B, C, H, W, N = 8, 256, 128, 128, 4
HW = H * W
EPS = 1e-5
P = 128
PT = 512              # p-tile width for pass 2
NPT = HW // PT        # 32 p-tiles
G1 = 8                # h-chunks per transpose group in pass 1
NG = P // G1          # 16 groups

_NC_CACHE = {}


def _band_consts():
    idx = np.arange(P)
    # erosion: output i covers input [i-8, i+4]; dilation: [i-4, i+8]
    band_er = ((idx[:, None] >= idx[None, :] - 8) &
               (idx[:, None] <= idx[None, :] + 4)).astype(np.float32)
    band_dl = ((idx[:, None] >= idx[None, :] - 4) &
               (idx[:, None] <= idx[None, :] + 8)).astype(np.float32)
    cnt_er = band_er.sum(axis=0, dtype=np.float32).reshape(P, 1)
    return band_er, band_dl, cnt_er


# packed fp32r parameter tensor "pk" [128, 1668]:
#   [0:512)     wo2t    : Wo2'^T chunks  [c-chunk ci -> cols ci*256:(ci+1)*256]
#   [512:1024)  wf_n    : Wf' blocks     [(ci*2+a)*128 ...] = Wf'[ci*128:,a*128:]
#   [1024:1152) identity (for the small-stage mid transpose)
#   [1152:1664) wo1t    : Wo1'^T blocks (ci*2+o)
#   [1664:1666) beta_f  (col ci)
#   [1666:1668) beta_o  (col o)
PK_W = 1668
# packed bf16 tensor "pkb" [128, 384]: band_er | band_dl | identity
PKB_W = 384
# packed fp32 tensor "pkf" [128, 513]: m (class-major cols) | cnt_er
PKF_W = 513


def build():
    if "nc" in _NC_CACHE:
        return _NC_CACHE["nc"]
    nc = bacc.Bacc(trn_type="TRN2", target_bir_lowering=False, debug=False)

    feature = nc.dram_tensor("feature", [C, HW], BF16, kind="ExternalInput")
    pk = nc.dram_tensor("pk", [P, PK_W], F32, kind="ExternalInput")
    pkb = nc.dram_tensor("pkb", [P, PKB_W], BF16, kind="ExternalInput")
    pkf = nc.dram_tensor("pkf", [P, PKF_W], F32, kind="ExternalInput")
    out = nc.dram_tensor("out", [C, HW], BF16, kind="ExternalOutput")

    with tile.TileContext(nc) as tc, ExitStack() as ctx:
        persist = ctx.enter_context(tc.tile_pool(name="persist", bufs=1))

        # ---------------- loads ----------------
        # everything the first transposes/morphology need rides the
        # low-latency sync (HWDGE) queue: identity+bands, then m, then the
        # first 2048 feature cols of each c-blk. Bulk feature on SWDGE.
        pkb_sb = persist.tile([P, PKB_W], BF16)
        nc.sync.dma_start(out=pkb_sb[:], in_=pkb[:])
        bander_sb = pkb_sb[:, 0:128]
        banddl_sb = pkb_sb[:, 128:256]
        ident_b = pkb_sb[:, 256:384]

        pkf_sb = persist.tile([P, PKF_W], F32)
        nc.sync.dma_start(out=pkf_sb[:], in_=pkf[:])
        m_sb = pkf_sb[:, 0:512]
        cnt_sb = pkf_sb[:, 512:513]

        feat = persist.tile([P, 2 * HW], BF16)   # c-blk0 | c-blk1
        for blk in range(2):
            nc.sync.dma_start(
                out=feat[:, blk * HW: blk * HW + 2048],
                in_=feature[blk * P:(blk + 1) * P, 0:2048])
        for blk in range(2):
            nc.gpsimd.dma_start(
                out=feat[:, blk * HW + 2048: blk * HW + 4096],
                in_=feature[blk * P:(blk + 1) * P, 2048:4096])
        for q in range(1, 4):
            for blk in range(2):
                nc.gpsimd.dma_start(
                    out=feat[:, blk * HW + q * 4096: blk * HW + (q + 1) * 4096],
                    in_=feature[blk * P:(blk + 1) * P, q * 4096:(q + 1) * 4096])

        # fp32r small-stage params last on the SWDGE queue (cast-loads are
        # gpsimd-only; needed only by the small stage ~40us in)
        pk_sb = persist.tile([P, PK_W], F32R)
        nc.gpsimd.dma_start(out=pk_sb[:], in_=pk[:])
        wo2_sb = pk_sb[:, 0:512]
        wfn_sb = pk_sb[:, 512:1024]
        ident_r = pk_sb[:, 1024:1152]
        wo1_sb = pk_sb[:, 1152:1664]
        betaf_sb = pk_sb[:, 1664:1666]
        betao_sb = pk_sb[:, 1666:1668]

        # persistent phase-1 state
        fbuT = persist.tile([P, 12 * P], BF16)     # k-major: [w, k*128 + h]
        ftall = persist.tile([P, 2 * HW], BF16)    # f^T: [w, h*256 + c]
        mid_sb = persist.tile([12, 256], F32R)
        wc_sb = persist.tile([P, 512], BF16)       # Wc^T blocks (a*2+o)
        u_sb = persist.tile([P, 2], F32)           # bias per o-blk

        wsrc = persist.tile([P, 512], BF16)
        z_sb = persist.tile([P, N * P], BF16)

        # ---------------- morphology + pass 1 (interleaved) ----------------
        with tc.tile_pool(name="morph", bufs=1) as mo, \
             tc.tile_pool(name="morph_ps", bufs=2, space="PSUM") as mops, \
             tc.tile_pool(name="mid_ps", bufs=1, space="PSUM") as midps, \
             tc.tile_pool(name="p1_ps", bufs=2, space="PSUM") as p1ps:

            def tr_group(g):
                # 16 PE transposes -> psum, one copy -> ftall
                tr = p1ps.tile([P, G1 * 256], BF16, tag="tr")
                for j in range(G1):
                    h = g * G1 + j
                    nc.tensor.matmul(tr[:, j * 256:j * 256 + P],
                                     feat[:, h * P:(h + 1) * P],
                                     ident_b, is_transpose=True)
                    nc.tensor.matmul(tr[:, j * 256 + P:(j + 1) * 256],
                                     feat[:, HW + h * P:HW + (h + 1) * P],
                                     ident_b, is_transpose=True)
                dst = ftall[:, g * G1 * 256:(g + 1) * G1 * 256]
                if g < 4 or g % 2 == 1:
                    nc.scalar.copy(dst, tr[:])
                else:
                    nc.vector.tensor_copy(dst, tr[:])

            mid_ps = midps.tile([12, 256], F32)
            fbuT_v = fbuT.rearrange("w (k h) -> w h k", k=12)

            def mid_group(g):
                for j in range(G1):
                    h = g * G1 + j
                    nc.tensor.matmul(mid_ps[:], fbuT_v[:, h, :],
                                     ftall[:, h * 256:(h + 1) * 256],
                                     start=(h == 0), stop=(h == P - 1),
                                     skip_group_check=True)

            # PE warm-keeper: bridges the startup window at cold clock so the
            # HAM gate is 8/8 when real work starts.
            nc.vector.memset(wsrc[:], 0.0)
            wp = mops.tile([P, 512], F32, tag="mps")
            for i in range(12):
                nc.tensor.matmul(wp[:], wsrc[:, 0:P], wsrc[:],
                                 start=True, stop=True, skip_group_check=True)
            nc.vector.tensor_scalar(z_sb[:], m_sb, 0.3, None, op0=ALU.is_gt)

            tr_group(0)
            tr_group(1)

            # morphology row pass
            ps_rows_er = mops.tile([P, N * P], F32, tag="mps")
            nc.tensor.matmul(ps_rows_er[:], bander_sb, z_sb[:],
                             start=True, stop=True)
            ps_rows_dl = mops.tile([P, N * P], F32, tag="mps")
            nc.tensor.matmul(ps_rows_dl[:], banddl_sb, z_sb[:],
                             start=True, stop=True)

            rows_er = mo.tile([P, N * P], BF16)
            nc.vector.tensor_scalar(rows_er[:], ps_rows_er[:], cnt_sb, None,
                                    op0=ALU.is_equal)
            rows_dl = mo.tile([P, N * P], BF16)
            nc.vector.tensor_scalar(rows_dl[:], ps_rows_dl[:], 0.5, None,
                                    op0=ALU.is_gt)

            tr_group(2)

            # transpose each class tile -> [w, h]
            rows_erT = mo.tile([P, N * P], BF16)
            rows_dlT = mo.tile([P, N * P], BF16)
            for n in range(N):
                ps_tr = mops.tile([P, 2 * P], BF16, tag="mps")
                nc.tensor.matmul(ps_tr[:, 0:P], rows_er[:, n * P:(n + 1) * P],
                                 ident_b, is_transpose=True)
                nc.tensor.matmul(ps_tr[:, P:2 * P], rows_dl[:, n * P:(n + 1) * P],
                                 ident_b, is_transpose=True)
                nc.vector.tensor_copy(rows_erT[:, n * P:(n + 1) * P], ps_tr[:, 0:P])
                nc.vector.tensor_copy(rows_dlT[:, n * P:(n + 1) * P], ps_tr[:, P:2 * P])

            tr_group(3)

            # morphology column pass
            ps_cols_er = mops.tile([P, N * P], F32, tag="mps")
            nc.tensor.matmul(ps_cols_er[:], bander_sb, rows_erT[:],
                             start=True, stop=True)
            ps_cols_dl = mops.tile([P, N * P], F32, tag="mps")
            nc.tensor.matmul(ps_cols_dl[:], banddl_sb, rows_dlT[:],
                             start=True, stop=True)

            tr_group(4)
            tr_group(5)

            # fbuT channels written straight from PSUM (k-major, contiguous)
            for n in range(N):
                pe = ps_cols_er[:, n * P:(n + 1) * P]
                pd = ps_cols_dl[:, n * P:(n + 1) * P]
                ch = lambda k: fbuT[:, k * P:(k + 1) * P]
                nc.vector.tensor_scalar(ch(3 * n), pe, cnt_sb, None,
                                        op0=ALU.is_equal)
                nc.vector.tensor_scalar(ch(3 * n + 1), pd, 0.0, None,
                                        op0=ALU.is_equal)
                nc.vector.scalar_tensor_tensor(ch(3 * n + 2), pd, 0.5,
                                               ch(3 * n), op0=ALU.is_gt,
                                               op1=ALU.subtract)

            tr_group(6)
            mid_group(0)
            for g in range(7, NG):
                tr_group(g)
                mid_group(g - 6)
            for g in range(NG - 6, NG):
                mid_group(g)

            nc.vector.tensor_copy(mid_sb[:], mid_ps[:])

        # ---------------- small stage: A'^T, Y, v -> X, u -> Wc ----------------
        with tc.tile_pool(name="sm_ps", bufs=1, space="PSUM") as smps, \
             tc.tile_pool(name="sm_sb", bufs=1) as smsb:
            # zero-dep dummies sprinkled through this stage keep the HAM
            # clock-gate at 8/8 (the serial engine hops otherwise idle the
            # PE long enough to re-throttle it before pass 2)
            wps = smps.tile([P, 512], F32, tag="wps")

            def keep_warm(n=2):
                for _ in range(n):
                    nc.tensor.matmul(wps[:], wsrc[:, 0:P], wsrc[:],
                                     start=True, stop=True,
                                     skip_group_check=True)

            # mid^T via PE transpose of [12,128] chunks (fp32r)
            ps_mt = smps.tile([P, 24], F32R, tag="mt")
            for ci in range(2):
                nc.tensor.matmul(ps_mt[:, ci * 12:(ci + 1) * 12],
                                 mid_sb[:, ci * P:(ci + 1) * P],
                                 pk_sb[0:12, 1024:1036], is_transpose=True)
            keep_warm()
            mid_t = smsb.tile([P, 24], F32R)
            nc.vector.tensor_copy(mid_t[:], ps_mt[:])

            # A'^T = mid @ Wo2'^T [12,256]; Y = mid @ Wf' [12,256]; v = mid @ bf
            ps_at = smps.tile([12, 256], F32, tag="at")
            ps_y = smps.tile([12, 256], F32, tag="y")
            ps_v = smps.tile([12, 2], F32, tag="v")
            for ci in range(2):
                nc.tensor.matmul(ps_at[:], mid_t[:, ci * 12:(ci + 1) * 12],
                                 wo2_sb[:, ci * 256:(ci + 1) * 256],
                                 start=(ci == 0), stop=(ci == 1))
            for a in range(2):
                for ci in range(2):
                    nc.tensor.matmul(ps_y[:, a * P:(a + 1) * P],
                                     mid_t[:, ci * 12:(ci + 1) * 12],
                                     wfn_sb[:, (ci * 2 + a) * P:(ci * 2 + a + 1) * P],
                                     start=(ci == 0), stop=(ci == 1),
                                     skip_group_check=True)
            # fp32r moving operands need an even column count: use a 2-col
            # sliding window over (beta_f | beta_o); only column 0 is the
            # real accumulation, column 1 is discarded.
            for ci in range(2):
                nc.tensor.matmul(ps_v[:], mid_t[:, ci * 12:(ci + 1) * 12],
                                 pk_sb[:, 1664 + ci:1666 + ci],
                                 start=(ci == 0), stop=(ci == 1))

            keep_warm()
            a_t = smsb.tile([12, 256], F32R)
            y_r = smsb.tile([12, 256], F32R)
            v_sb = smsb.tile([12, 2], F32R)
            nc.vector.tensor_copy(a_t[:], ps_at[:])
            nc.scalar.copy(y_r[:], ps_y[:])
            nc.vector.tensor_copy(v_sb[:], ps_v[:])
            keep_warm()

            # X = Y^T @ A'^T  (blocks a on partitions);  u' = A' v
            ps_x = smps.tile([P, 512], F32, tag="x")
            for a in range(2):
                nc.tensor.matmul(ps_x[:, a * 256:(a + 1) * 256],
                                 y_r[:, a * P:(a + 1) * P], a_t[:],
                                 start=True, stop=True, skip_group_check=True)
            ps_u = smps.tile([P, 4], F32, tag="u")
            for o in range(2):
                nc.tensor.matmul(ps_u[:, 2 * o:2 * o + 2],
                                 a_t[:, o * P:(o + 1) * P], v_sb[:],
                                 start=True, stop=True, skip_group_check=True)
            keep_warm(3)

            # Wc^T = Wo1'^T + X;  u = u' + beta_o
            for a in range(2):
                nc.vector.tensor_tensor(wc_sb[:, a * 256:(a + 1) * 256],
                                        ps_x[:, a * 256:(a + 1) * 256],
                                        wo1_sb[:, a * 256:(a + 1) * 256],
                                        op=ALU.add)
            for o in range(2):
                nc.scalar.activation(u_sb[:, o:o + 1], ps_u[:, 2 * o:2 * o + 1],
                                     ACTF.Identity, bias=betao_sb[:, o:o + 1])

        # ---------------- pass 2: out = Wc @ f + u ----------------
        with tc.tile_pool(name="out_ps", bufs=4, space="PSUM") as outps, \
             tc.tile_pool(name="p2_sb", bufs=3) as p2sb:
            for tg in range(NPT // 4):
                ot0 = p2sb.tile([P, 4 * PT], BF16, tag="ot0")
                ot1 = p2sb.tile([P, 4 * PT], BF16, tag="ot1")
                for tt in range(4):
                    t = tg * 4 + tt
                    c0 = t * PT
                    out_ps = outps.tile([P, 2 * PT], F32, tag="ops")
                    for o in range(2):
                        ops = out_ps[:, o * PT:(o + 1) * PT]
                        nc.tensor.matmul(ops,
                                         wc_sb[:, (0 * 2 + o) * P:(0 * 2 + o + 1) * P],
                                         feat[:, c0:c0 + PT],
                                         start=True, stop=False, skip_group_check=True)
                        nc.tensor.matmul(ops,
                                         wc_sb[:, (1 * 2 + o) * P:(1 * 2 + o + 1) * P],
                                         feat[:, HW + c0:HW + c0 + PT],
                                         start=False, stop=True, skip_group_check=True)
                    nc.scalar.activation(ot0[:, tt * PT:(tt + 1) * PT],
                                         out_ps[:, 0:PT],
                                         ACTF.Identity, bias=u_sb[:, 0:1])
                    nc.vector.tensor_scalar(ot1[:, tt * PT:(tt + 1) * PT],
                                            out_ps[:, PT:2 * PT],
                                            u_sb[:, 1:2], None, op0=ALU.add)
                g0 = tg * 4 * PT
                nc.sync.dma_start(out=out[0:P, g0:g0 + 4 * PT], in_=ot0[:])
                nc.gpsimd.dma_start(out=out[P:C, g0:g0 + 4 * PT], in_=ot1[:])

    nc.compile()
    _NC_CACHE["nc"] = nc
    return nc


def prepare_in_maps(feature, m, W_f, g_f, b_f, mu_f, v_f, W_o, g_o, b_o, mu_o, v_o):
    feature = np.asarray(feature, dtype=np.float32)
    m = np.asarray(m, dtype=np.float32)
    W_f = np.asarray(W_f, dtype=np.float32)
    W_o = np.asarray(W_o, dtype=np.float32)
    g_f, b_f, mu_f, v_f = (np.asarray(x, dtype=np.float32) for x in (g_f, b_f, mu_f, v_f))
    g_o, b_o, mu_o, v_o = (np.asarray(x, dtype=np.float32) for x in (g_o, b_o, mu_o, v_o))

    inv_f = g_f / np.sqrt(v_f + EPS)
    beta_f_v = b_f - mu_f * inv_f
    inv_o = g_o / np.sqrt(v_o + EPS)
    beta_o_v = b_o - mu_o * inv_o
    Wf_p = (inv_f[:, None] * W_f).astype(np.float32)          # [C, C]
    Wo1_p = (inv_o[:, None] * W_o[:, :C]).astype(np.float32)  # [C, C]
    Wo2_p = (inv_o[:, None] * W_o[:, C:]).astype(np.float32)  # [C, C]

    def blocks_t(Wp):
        # lhsT layout: blocks ci*2+o of Wp^T
        a = np.empty((P, 512), np.float32)
        for ci in range(2):
            for o in range(2):
                a[:, (ci * 2 + o) * P:(ci * 2 + o + 1) * P] = \
                    Wp[o * P:(o + 1) * P, ci * P:(ci + 1) * P].T
        return a

    def blocks_n(Wp):
        # natural-layout blocks ci*2+a: Wp[ci*128:(ci+1)*128, a*128:(a+1)*128]
        a_ = np.empty((P, 512), np.float32)
        for ci in range(2):
            for a in range(2):
                a_[:, (ci * 2 + a) * P:(ci * 2 + a + 1) * P] = \
                    Wp[ci * P:(ci + 1) * P, a * P:(a + 1) * P]
        return a_

    band_er, band_dl, cnt_er = _band_consts()
    pk = np.empty((P, PK_W), np.float32)
    pk[:, 0:512] = np.concatenate([Wo2_p.T[0:P, :], Wo2_p.T[P:C, :]], axis=1)
    pk[:, 512:1024] = blocks_n(Wf_p)
    pk[:, 1024:1152] = np.eye(P, dtype=np.float32)
    pk[:, 1152:1664] = blocks_t(Wo1_p)
    pk[:, 1664:1666] = beta_f_v.reshape(2, P).T
    pk[:, 1666:1668] = beta_o_v.reshape(2, P).T

    pkb = np.empty((P, PKB_W), np.float32)
    pkb[:, 0:128] = band_er
    pkb[:, 128:256] = band_dl
    pkb[:, 256:384] = np.eye(P, dtype=np.float32)
    pkb = pkb.astype(ml_dtypes.bfloat16)

    pkf = np.empty((P, PKF_W), np.float32)
    pkf[:, 512:513] = cnt_er

    feat_b = feature.reshape(B, C, HW).astype(ml_dtypes.bfloat16)

    in_maps = []
    for b in range(B):
        im = {"pk": pk, "pkb": pkb}
        pkf_b = pkf.copy()
        # m per class into columns [n*128:(n+1)*128]
        pkf_b[:, 0:512] = np.transpose(m[b], (1, 0, 2)).reshape(P, 512)
        im["pkf"] = pkf_b
        im["feature"] = np.ascontiguousarray(feat_b[b])
        in_maps.append(im)
    return in_maps


def kernel(feature, m, W_f, g_f, b_f, mu_f, v_f, W_o, g_o, b_o, mu_o, v_o):
    nc = build()
    in_maps = prepare_in_maps(feature, m, W_f, g_f, b_f, mu_f, v_f,
                              W_o, g_o, b_o, mu_o, v_o)
    res = bass_utils.run_bass_kernel_spmd(nc, in_maps, list(range(B)))
    out = np.empty((B, C, H, W), np.float32)
    for b in range(B):
        out[b] = np.asarray(res.results[b]["out"], dtype=np.float32).reshape(C, H, W)
    return out


# revision 29
# speedup vs baseline: 1.1157x; 1.1157x over previous
"""Trainium2 Bass kernel for nn_BDFM_Multi (B=8,C=256,H=W=128,N=4).

Data-parallel over batch: one batch element per NeuronCore (8 cores).

Per-core computation (feature f [C,HW], m [N,H,W], HW=16384):
  z    = (m > 0.3)                                  binary
  er   = 13-tap separable min-filter(z), dl = 13-tap separable max-filter(z)
         -> banded 0/1 matmuls + thresholds (exact on binary data)
  fbu  = per-class channels (er, 1-dl, dl-er)       [12, HW]
  mid  = fbu @ f^T                                  [12, C]
  A'^T = mid @ Wo2'^T, Y = mid @ Wf', v = mid @ beta_f
  X    = Y^T @ A'^T  (= (G Wf')^T = Wf'^T G^T)
  Wc^T = Wo1'^T + X;  u = A'^T^T v + beta_o
  out  = Wc @ f + u                                 [C, HW]
  (exact algebraic refactor of out = BN(Wo @ [f; mid^T @ (mid @ BN(Wf@f))]))

v3: bf16 feature/out streams; transposes staggered around morphology with a
fully persistent f^T buffer; k-major fbu^T built directly from PSUM; the
small [C,C] stage runs in 7 engine hops (float32r).
"""
import numpy as np
import ml_dtypes
from contextlib import ExitStack

import concourse.bass as bass
import concourse.mybir as mybir
import concourse.tile as tile
from concourse import bacc
from concourse import bass_utils

F32 = mybir.dt.float32
F32R = mybir.dt.float32r
BF16 = mybir.dt.bfloat16
ALU = mybir.AluOpType
ACTF = mybir.ActivationFunctionType

B, C, H, W, N = 8, 256, 128, 128, 4
HW = H * W
EPS = 1e-5
P = 128
PT = 512              # p-tile width for pass 2
NPT = HW // PT        # 32 p-tiles
G1 = 8                # h-chunks per transpose group in pass 1
NG = P // G1          # 16 groups

_NC_CACHE = {}


def _band_consts():
    idx = np.arange(P)
    # erosion: output i covers input [i-8, i+4]; dilation: [i-4, i+8]
    band_er = ((idx[:, None] >= idx[None, :] - 8) &
               (idx[:, None] <= idx[None, :] + 4)).astype(np.float32)
    band_dl = ((idx[:, None] >= idx[None, :] - 4) &
               (idx[:, None] <= idx[None, :] + 8)).astype(np.float32)
    cnt_er = band_er.sum(axis=0, dtype=np.float32).reshape(P, 1)
    return band_er, band_dl, cnt_er


# packed fp32r parameter tensor "pk" [128, 1668]:
#   [0:512)     wo2t    : Wo2'^T chunks  [c-chunk ci -> cols ci*256:(ci+1)*256]
#   [512:1024)  wf_n    : Wf' blocks     [(ci*2+a)*128 ...] = Wf'[ci*128:,a*128:]
#   [1024:1152) identity (for the small-stage mid transpose)
#   [1152:1664) wo1t    : Wo1'^T blocks (ci*2+o)
#   [1664:1666) beta_f  (col ci)
#   [1666:1668) beta_o  (col o)
PK_W = 1668
# packed bf16 tensor "pkb" [128, 384]: band_er | band_dl | identity
PKB_W = 384
# packed fp32 tensor "pkf" [128, 513]: m (class-major cols) | cnt_er
PKF_W = 513


def build():
    if "nc" in _NC_CACHE:
        return _NC_CACHE["nc"]
    nc = bacc.Bacc(trn_type="TRN2", target_bir_lowering=False, debug=False)

    feature = nc.dram_tensor("feature", [C, HW], BF16, kind="ExternalInput")
    pk = nc.dram_tensor("pk", [P, PK_W], F32, kind="ExternalInput")
    pkb = nc.dram_tensor("pkb", [P, PKB_W], BF16, kind="ExternalInput")
    pkf = nc.dram_tensor("pkf", [P, PKF_W], F32, kind="ExternalInput")
    out = nc.dram_tensor("out", [C, HW], BF16, kind="ExternalOutput")

    with tile.TileContext(nc) as tc, ExitStack() as ctx:
        persist = ctx.enter_context(tc.tile_pool(name="persist", bufs=1))

        # ---------------- loads ----------------
        # morphology-critical params first on the SWDGE queue
        pkf_sb = persist.tile([P, PKF_W], F32)
        nc.gpsimd.dma_start(out=pkf_sb[:], in_=pkf[:])
        m_sb = pkf_sb[:, 0:512]
        cnt_sb = pkf_sb[:, 512:513]

        pkb_sb = persist.tile([P, PKB_W], BF16)
        nc.gpsimd.dma_start(out=pkb_sb[:], in_=pkb[:])
        bander_sb = pkb_sb[:, 0:128]
        banddl_sb = pkb_sb[:, 128:256]
        ident_b = pkb_sb[:, 256:384]

        # feature: first q-chunk split in half so transposes unblock earlier
        feat = persist.tile([P, 2 * HW], BF16)   # c-blk0 | c-blk1
        for half in range(2):
            for blk in range(2):
                c0 = half * 2048
                nc.gpsimd.dma_start(
                    out=feat[:, blk * HW + c0: blk * HW + c0 + 2048],
                    in_=feature[blk * P:(blk + 1) * P, c0:c0 + 2048])
        for q in range(1, 4):
            for blk in range(2):
                nc.gpsimd.dma_start(
                    out=feat[:, blk * HW + q * 4096: blk * HW + (q + 1) * 4096],
                    in_=feature[blk * P:(blk + 1) * P, q * 4096:(q + 1) * 4096])

        # fp32r small-stage params last on the SWDGE queue (cast-loads are
        # gpsimd-only; needed only by the small stage ~40us in)
        pk_sb = persist.tile([P, PK_W], F32R)
        nc.gpsimd.dma_start(out=pk_sb[:], in_=pk[:])
        wo2_sb = pk_sb[:, 0:512]
        wfn_sb = pk_sb[:, 512:1024]
        ident_r = pk_sb[:, 1024:1152]
        wo1_sb = pk_sb[:, 1152:1664]
        betaf_sb = pk_sb[:, 1664:1666]
        betao_sb = pk_sb[:, 1666:1668]

        # persistent phase-1 state
        fbuT = persist.tile([P, 12 * P], BF16)     # k-major: [w, k*128 + h]
        ftall = persist.tile([P, 2 * HW], BF16)    # f^T: [w, h*256 + c]
        mid_sb = persist.tile([12, 256], F32R)
        wc_sb = persist.tile([P, 512], BF16)       # Wc^T blocks (a*2+o)
        u_sb = persist.tile([P, 2], F32)           # bias per o-blk

        wsrc = persist.tile([P, 512], BF16)
        z_sb = persist.tile([P, N * P], BF16)

        # ---------------- morphology + pass 1 (interleaved) ----------------
        with tc.tile_pool(name="morph", bufs=1) as mo, \
             tc.tile_pool(name="morph_ps", bufs=2, space="PSUM") as mops, \
             tc.tile_pool(name="mid_ps", bufs=1, space="PSUM") as midps, \
             tc.tile_pool(name="p1_ps", bufs=2, space="PSUM") as p1ps:

            def tr_group(g):
                # 16 PE transposes -> psum, one copy -> ftall
                tr = p1ps.tile([P, G1 * 256], BF16, tag="tr")
                for j in range(G1):
                    h = g * G1 + j
                    nc.tensor.matmul(tr[:, j * 256:j * 256 + P],
                                     feat[:, h * P:(h + 1) * P],
                                     ident_b, is_transpose=True)
                    nc.tensor.matmul(tr[:, j * 256 + P:(j + 1) * 256],
                                     feat[:, HW + h * P:HW + (h + 1) * P],
                                     ident_b, is_transpose=True)
                dst = ftall[:, g * G1 * 256:(g + 1) * G1 * 256]
                if g < 4 or g % 2 == 1:
                    nc.scalar.copy(dst, tr[:])
                else:
                    nc.vector.tensor_copy(dst, tr[:])

            mid_ps = midps.tile([12, 256], F32)
            fbuT_v = fbuT.rearrange("w (k h) -> w h k", k=12)

            def mid_group(g):
                for j in range(G1):
                    h = g * G1 + j
                    nc.tensor.matmul(mid_ps[:], fbuT_v[:, h, :],
                                     ftall[:, h * 256:(h + 1) * 256],
                                     start=(h == 0), stop=(h == P - 1),
                                     skip_group_check=True)

            # PE warm-keeper: bridges the startup window at cold clock so the
            # HAM gate is 8/8 when real work starts.
            nc.vector.memset(wsrc[:], 0.0)
            wp = mops.tile([P, 512], F32, tag="mps")
            for i in range(10):
                nc.tensor.matmul(wp[:], wsrc[:, 0:P], wsrc[:],
                                 start=True, stop=True, skip_group_check=True)
            nc.vector.tensor_scalar(z_sb[:], m_sb, 0.3, None, op0=ALU.is_gt)

            tr_group(0)
            tr_group(1)

            # morphology row pass
            ps_rows_er = mops.tile([P, N * P], F32, tag="mps")
            nc.tensor.matmul(ps_rows_er[:], bander_sb, z_sb[:],
                             start=True, stop=True)
            ps_rows_dl = mops.tile([P, N * P], F32, tag="mps")
            nc.tensor.matmul(ps_rows_dl[:], banddl_sb, z_sb[:],
                             start=True, stop=True)

            rows_er = mo.tile([P, N * P], BF16)
            nc.vector.tensor_scalar(rows_er[:], ps_rows_er[:], cnt_sb, None,
                                    op0=ALU.is_equal)
            rows_dl = mo.tile([P, N * P], BF16)
            nc.vector.tensor_scalar(rows_dl[:], ps_rows_dl[:], 0.5, None,
                                    op0=ALU.is_gt)

            tr_group(2)

            # transpose each class tile -> [w, h]
            rows_erT = mo.tile([P, N * P], BF16)
            rows_dlT = mo.tile([P, N * P], BF16)
            for n in range(N):
                ps_tr = mops.tile([P, 2 * P], BF16, tag="mps")
                nc.tensor.matmul(ps_tr[:, 0:P], rows_er[:, n * P:(n + 1) * P],
                                 ident_b, is_transpose=True)
                nc.tensor.matmul(ps_tr[:, P:2 * P], rows_dl[:, n * P:(n + 1) * P],
                                 ident_b, is_transpose=True)
                nc.vector.tensor_copy(rows_erT[:, n * P:(n + 1) * P], ps_tr[:, 0:P])
                nc.vector.tensor_copy(rows_dlT[:, n * P:(n + 1) * P], ps_tr[:, P:2 * P])

            tr_group(3)

            # morphology column pass
            ps_cols_er = mops.tile([P, N * P], F32, tag="mps")
            nc.tensor.matmul(ps_cols_er[:], bander_sb, rows_erT[:],
                             start=True, stop=True)
            ps_cols_dl = mops.tile([P, N * P], F32, tag="mps")
            nc.tensor.matmul(ps_cols_dl[:], banddl_sb, rows_dlT[:],
                             start=True, stop=True)

            tr_group(4)
            tr_group(5)

            # fbuT channels written straight from PSUM (k-major, contiguous)
            for n in range(N):
                pe = ps_cols_er[:, n * P:(n + 1) * P]
                pd = ps_cols_dl[:, n * P:(n + 1) * P]
                ch = lambda k: fbuT[:, k * P:(k + 1) * P]
                nc.vector.tensor_scalar(ch(3 * n), pe, cnt_sb, None,
                                        op0=ALU.is_equal)
                nc.vector.tensor_scalar(ch(3 * n + 1), pd, 0.0, None,
                                        op0=ALU.is_equal)
                nc.vector.scalar_tensor_tensor(ch(3 * n + 2), pd, 0.5,
                                               ch(3 * n), op0=ALU.is_gt,
                                               op1=ALU.subtract)

            tr_group(6)
            mid_group(0)
            for g in range(7, NG):
                tr_group(g)
                mid_group(g - 6)
            for g in range(NG - 6, NG):
                mid_group(g)

            nc.vector.tensor_copy(mid_sb[:], mid_ps[:])

        # ---------------- small stage: A'^T, Y, v -> X, u -> Wc ----------------
        with tc.tile_pool(name="sm_ps", bufs=1, space="PSUM") as smps, \
             tc.tile_pool(name="sm_sb", bufs=1) as smsb:
            # mid^T via PE transpose of [12,128] chunks (fp32r)
            ps_mt = smps.tile([P, 24], F32R, tag="mt")
            for ci in range(2):
                nc.tensor.matmul(ps_mt[:, ci * 12:(ci + 1) * 12],
                                 mid_sb[:, ci * P:(ci + 1) * P],
                                 pk_sb[0:12, 1024:1036], is_transpose=True)
            mid_t = smsb.tile([P, 24], F32R)
            nc.vector.tensor_copy(mid_t[:], ps_mt[:])

            # A'^T = mid @ Wo2'^T [12,256]; Y = mid @ Wf' [12,256]; v = mid @ bf
            ps_at = smps.tile([12, 256], F32, tag="at")
            ps_y = smps.tile([12, 256], F32, tag="y")
            ps_v = smps.tile([12, 2], F32, tag="v")
            for ci in range(2):
                nc.tensor.matmul(ps_at[:], mid_t[:, ci * 12:(ci + 1) * 12],
                                 wo2_sb[:, ci * 256:(ci + 1) * 256],
                                 start=(ci == 0), stop=(ci == 1))
            for a in range(2):
                for ci in range(2):
                    nc.tensor.matmul(ps_y[:, a * P:(a + 1) * P],
                                     mid_t[:, ci * 12:(ci + 1) * 12],
                                     wfn_sb[:, (ci * 2 + a) * P:(ci * 2 + a + 1) * P],
                                     start=(ci == 0), stop=(ci == 1),
                                     skip_group_check=True)
            # fp32r moving operands need an even column count: use a 2-col
            # sliding window over (beta_f | beta_o); only column 0 is the
            # real accumulation, column 1 is discarded.
            for ci in range(2):
                nc.tensor.matmul(ps_v[:], mid_t[:, ci * 12:(ci + 1) * 12],
                                 pk_sb[:, 1664 + ci:1666 + ci],
                                 start=(ci == 0), stop=(ci == 1))

            a_t = smsb.tile([12, 256], F32R)
            y_r = smsb.tile([12, 256], F32R)
            v_sb = smsb.tile([12, 2], F32R)
            nc.vector.tensor_copy(a_t[:], ps_at[:])
            nc.scalar.copy(y_r[:], ps_y[:])
            nc.vector.tensor_copy(v_sb[:], ps_v[:])

            # X = Y^T @ A'^T  (blocks a on partitions);  u' = A' v
            ps_x = smps.tile([P, 512], F32, tag="x")
            for a in range(2):
                nc.tensor.matmul(ps_x[:, a * 256:(a + 1) * 256],
                                 y_r[:, a * P:(a + 1) * P], a_t[:],
                                 start=True, stop=True, skip_group_check=True)
            ps_u = smps.tile([P, 4], F32, tag="u")
            for o in range(2):
                nc.tensor.matmul(ps_u[:, 2 * o:2 * o + 2],
                                 a_t[:, o * P:(o + 1) * P], v_sb[:],
                                 start=True, stop=True, skip_group_check=True)

            # Wc^T = Wo1'^T + X;  u = u' + beta_o
            for a in range(2):
                nc.vector.tensor_tensor(wc_sb[:, a * 256:(a + 1) * 256],
                                        ps_x[:, a * 256:(a + 1) * 256],
                                        wo1_sb[:, a * 256:(a + 1) * 256],
                                        op=ALU.add)
            for o in range(2):
                nc.scalar.activation(u_sb[:, o:o + 1], ps_u[:, 2 * o:2 * o + 1],
                                     ACTF.Identity, bias=betao_sb[:, o:o + 1])

        # ---------------- pass 2: out = Wc @ f + u ----------------
        with tc.tile_pool(name="out_ps", bufs=4, space="PSUM") as outps, \
             tc.tile_pool(name="p2_sb", bufs=2) as p2sb:
            for tg in range(NPT // 4):
                ot0 = p2sb.tile([P, 4 * PT], BF16, tag="ot0")
                ot1 = p2sb.tile([P, 4 * PT], BF16, tag="ot1")
                for tt in range(4):
                    t = tg * 4 + tt
                    c0 = t * PT
                    out_ps = outps.tile([P, 2 * PT], F32, tag="ops")
                    for o in range(2):
                        ops = out_ps[:, o * PT:(o + 1) * PT]
                        nc.tensor.matmul(ops,
                                         wc_sb[:, (0 * 2 + o) * P:(0 * 2 + o + 1) * P],
                                         feat[:, c0:c0 + PT],
                                         start=True, stop=False, skip_group_check=True)
                        nc.tensor.matmul(ops,
                                         wc_sb[:, (1 * 2 + o) * P:(1 * 2 + o + 1) * P],
                                         feat[:, HW + c0:HW + c0 + PT],
                                         start=False, stop=True, skip_group_check=True)
                    nc.scalar.activation(ot0[:, tt * PT:(tt + 1) * PT],
                                         out_ps[:, 0:PT],
                                         ACTF.Identity, bias=u_sb[:, 0:1])
                    nc.vector.tensor_scalar(ot1[:, tt * PT:(tt + 1) * PT],
                                            out_ps[:, PT:2 * PT],
                                            u_sb[:, 1:2], None, op0=ALU.add)
                g0 = tg * 4 * PT
                nc.sync.dma_start(out=out[0:P, g0:g0 + 4 * PT], in_=ot0[:])
                nc.gpsimd.dma_start(out=out[P:C, g0:g0 + 4 * PT], in_=ot1[:])

    nc.compile()
    _NC_CACHE["nc"] = nc
    return nc


def prepare_in_maps(feature, m, W_f, g_f, b_f, mu_f, v_f, W_o, g_o, b_o, mu_o, v_o):
    feature = np.asarray(feature, dtype=np.float32)
    m = np.asarray(m, dtype=np.float32)
    W_f = np.asarray(W_f, dtype=np.float32)
    W_o = np.asarray(W_o, dtype=np.float32)
    g_f, b_f, mu_f, v_f = (np.asarray(x, dtype=np.float32) for x in (g_f, b_f, mu_f, v_f))
    g_o, b_o, mu_o, v_o = (np.asarray(x, dtype=np.float32) for x in (g_o, b_o, mu_o, v_o))

    inv_f = g_f / np.sqrt(v_f + EPS)
    beta_f_v = b_f - mu_f * inv_f
    inv_o = g_o / np.sqrt(v_o + EPS)
    beta_o_v = b_o - mu_o * inv_o
    Wf_p = (inv_f[:, None] * W_f).astype(np.float32)          # [C, C]
    Wo1_p = (inv_o[:, None] * W_o[:, :C]).astype(np.float32)  # [C, C]
    Wo2_p = (inv_o[:, None] * W_o[:, C:]).astype(np.float32)  # [C, C]

    def blocks_t(Wp):
        # lhsT layout: blocks ci*2+o of Wp^T
        a = np.empty((P, 512), np.float32)
        for ci in range(2):
            for o in range(2):
                a[:, (ci * 2 + o) * P:(ci * 2 + o + 1) * P] = \
                    Wp[o * P:(o + 1) * P, ci * P:(ci + 1) * P].T
        return a

    def blocks_n(Wp):
        # natural-layout blocks ci*2+a: Wp[ci*128:(ci+1)*128, a*128:(a+1)*128]
        a_ = np.empty((P, 512), np.float32)
        for ci in range(2):
            for a in range(2):
                a_[:, (ci * 2 + a) * P:(ci * 2 + a + 1) * P] = \
                    Wp[ci * P:(ci + 1) * P, a * P:(a + 1) * P]
        return a_

    band_er, band_dl, cnt_er = _band_consts()
    pk = np.empty((P, PK_W), np.float32)
    pk[:, 0:512] = np.concatenate([Wo2_p.T[0:P, :], Wo2_p.T[P:C, :]], axis=1)
    pk[:, 512:1024] = blocks_n(Wf_p)
    pk[:, 1024:1152] = np.eye(P, dtype=np.float32)
    pk[:, 1152:1664] = blocks_t(Wo1_p)
    pk[:, 1664:1666] = beta_f_v.reshape(2, P).T
    pk[:, 1666:1668] = beta_o_v.reshape(2, P).T

    pkb = np.empty((P, PKB_W), np.float32)
    pkb[:, 0:128] = band_er
    pkb[:, 128:256] = band_dl
    pkb[:, 256:384] = np.eye(P, dtype=np.float32)
    pkb = pkb.astype(ml_dtypes.bfloat16)

    pkf = np.empty((P, PKF_W), np.float32)
    pkf[:, 512:513] = cnt_er

    feat_b = feature.reshape(B, C, HW).astype(ml_dtypes.bfloat16)

    in_maps = []
    for b in range(B):
        im = {"pk": pk, "pkb": pkb}
        pkf_b = pkf.copy()
        # m per class into columns [n*128:(n+1)*128]
        pkf_b[:, 0:512] = np.transpose(m[b], (1, 0, 2)).reshape(P, 512)
        im["pkf"] = pkf_b
        im["feature"] = np.ascontiguousarray(feat_b[b])
        in_maps.append(im)
    return in_maps


def kernel(feature, m, W_f, g_f, b_f, mu_f, v_f, W_o, g_o, b_o, mu_o, v_o):
    nc = build()
    in_maps = prepare_in_maps(feature, m, W_f, g_f, b_f, mu_f, v_f,
                              W_o, g_o, b_o, mu_o, v_o)
    res = bass_utils.run_bass_kernel_spmd(nc, in_maps, list(range(B)))
    out = np.empty((B, C, H, W), np.float32)
    for b in range(B):
        out[b] = np.asarray(res.results[b]["out"], dtype=np.float32).reshape(C, H, W)
    return out
